# revision 38
# baseline (speedup 1.0000x reference)
"""Trainium2 Bass kernel for nn_Attention_12146167513140.

Distributed dense attention over 8 NeuronCores.

Sharding: core c in 0..7 -> (b = c//4, head-pair hp = c%4).  Each core
computes the full [3072 q x 3072 k] attention for its 2 heads of its
batch, producing a partial output projection [3072, 256]; the host sums
the 4 partials per batch and adds b_out.

Device pipeline per core (all matmuls bf16, accumulation f32 in PSUM):
  A) kv = s2 @ Wkv_pair -> rms-norm k -> kT2 (PE transpose),
     v -> vx tiles (4 sub-tiles of 17 cols: 16 v dims + ones col for Z)
  B) q = s1e @ Wq_pair -> rms-norm q -> qT2
  C) flash-style, PE-array tiled:
     - QK: 2-way row tiling (K=33: 32 dims + mask row).  Head 0 in array
       rows 0-63, head 1 in rows 64-127, concurrent.
     - exp: dispatched per key-chunk to ScalarE (exact exp, scale fused)
       or DVE via a 1-op Schraudolph fast-exp (f32 mult-add with int16
       writeback; the int16 bit pattern IS the bf16 exp approximation;
       RNE + saturation verified on HW).  Pattern "ADA" (period 3) pins
       one engine per sc-ring slot.  Mask bias is -30/SCALE so fast-exp
       inputs stay in the int16-safe range.
     - PV: 2-way column tiling (M=33 = ones|v per head); the leading
       ones column accumulates the softmax denominator Z at aligned
       PSUM partitions 0/64.
  D) 1/Z via tiny PE transposes of the Z rows already in oT_sb + DVE
     reciprocal; out_partial = (oT/Z).T @ Wout (zero-padded rows drop
     the Z row and match the oT_sb layout), 2-way row-tiled.

Host-side prep: sinusoidal positional embedding (index arithmetic),
transposes, bf16 casts, mask row encoding.
"""

import contextlib
import ctypes
import sys
import types

import numpy as np
import ml_dtypes

import concourse.bacc as bacc
import concourse.mybir as mybir
from concourse import bass_utils
from concourse.tile import TileContext
from concourse.alu_op_type import AluOpType
from concourse.mybir import ActivationFunctionType as AF


def _ensure_trace_support():
    """The container's antenv package lacks axon_hooks; bass_utils
    imports it when tracing is requested (e.g. via BASS_TRACE).  Install
    a functional shim so a traced run works instead of crashing, and
    make the artifact upload a no-op (no bucket access here)."""
    try:
        import antenv.axon_hooks  # noqa: F401
        return
    except ImportError:
        pass
    mod = types.ModuleType("antenv.axon_hooks")
    mod._hook = None
    mod.set_axon_ntff_profile_hook = lambda h: setattr(mod, "_hook", h)
    mod.get_axon_ntff_profile_hook = lambda: mod._hook
    try:
        import antenv
        sys.modules["antenv.axon_hooks"] = mod
        antenv.axon_hooks = mod
    except ImportError:
        sys.modules["antenv.axon_hooks"] = mod

    def _ntff_hook(so_path):
        try:
            lib = ctypes.CDLL(so_path)
        except OSError:
            return None
        if not hasattr(lib, "axon_start_nrt_profile"):
            return None
        lib.axon_start_nrt_profile.argtypes = [ctypes.POINTER(ctypes.c_int64),
                                               ctypes.c_size_t]
        lib.axon_start_nrt_profile.restype = ctypes.c_int64
        lib.axon_stop_nrt_profile.argtypes = [ctypes.c_char_p]
        lib.axon_stop_nrt_profile.restype = ctypes.c_int64

        @contextlib.contextmanager
        def _hook(output_dir, device_ids):
            import jax
            jax.devices()
            if device_ids:
                ids = (ctypes.c_int64 * len(device_ids))(*device_ids)
                rc = lib.axon_start_nrt_profile(ids, len(device_ids))
            else:
                rc = lib.axon_start_nrt_profile(None, 0)
            if rc != 0:
                raise RuntimeError(f"axon_start_nrt_profile rc={rc}")
            try:
                yield
            finally:
                lib.axon_stop_nrt_profile(str(output_dir).encode())

        return _hook

    mod.set_axon_ntff_profile_hook(_ntff_hook("/opt/axon/libaxon_pjrt.so"))

    _orig_upload = bass_utils.upload_artifacts

    def _safe_upload(tmpdir):
        try:
            return _orig_upload(tmpdir)
        except Exception:
            return tmpdir

    bass_utils.upload_artifacts = _safe_upload


_ensure_trace_support()

AX = mybir.AxisListType
I16 = mybir.dt.int16
I32 = mybir.dt.int32
BF = mybir.dt.bfloat16
F32 = mybir.dt.float32
bf16 = ml_dtypes.bfloat16

B, N1, N2 = 2, 3072, 3072
C_S, H, D = 256, 8, 32
INF = 100000.0
EPS = 1e-8
SCALE = float(np.sqrt(1.0 / (3 * D)))
MASKV = 30.0          # effective -30 in the exp argument for masked keys

NCORES = 8
HPC = 2            # heads per core
KCH = N2 // 128    # 24 key chunks
QCH = N1 // 128    # 24 q row chunks
QB = 512           # q block for scores free dim
NQB = N1 // QB     # 6
VJ = 17            # PV col-tile width: 16 v dims + ones col

# Schraudolph fast-exp constants: int16(y*AS + BS) viewed as bf16 ~ exp(y)
# for y = raw score (SCALE folded into AS).  Calibrated on HW (RNE writeback).
LN2 = float(np.log(2.0))
AS_EXP = SCALE * 128.0 / LN2
BS_EXP = 127.0 * 128.0 - 7.4

import os
# exp engine dispatch patterns (per key chunk index):
# A = ScalarE exact exp, D = DVE fast-exp (GPSIMD cannot read PSUM)
PAT_EARLY = os.environ.get("KPE", "A")    # qb 0-1: ScalarE only; DVE runs prologue fillers
PAT_MAIN = os.environ.get("KPM", "ADA")   # qb 2+: period 3 = one engine per ring slot

_cache = {}
KBISECT = int(os.environ.get("KBISECT", "5"))


def _build(use_g2: bool):
    nc = bacc.Bacc("TRN2", target_bir_lowering=False, debug=False, num_devices=NCORES)

    s1T_d = nc.dram_tensor("s1T", [C_S, N1], BF, kind="ExternalInput")
    s2T_d = nc.dram_tensor("s2T", [C_S, N2], BF, kind="ExternalInput")
    wq_d = nc.dram_tensor("wq", [C_S, HPC * D], BF, kind="ExternalInput")
    wkv_d = nc.dram_tensor("wkv", [C_S, HPC * 2 * D], BF, kind="ExternalInput")
    wout_d = nc.dram_tensor("wout", [97, C_S], BF, kind="ExternalInput")
    qm_d = nc.dram_tensor("qm", [1, N1], BF, kind="ExternalInput")
    km_d = nc.dram_tensor("km", [1, N2], BF, kind="ExternalInput")
    id_d = nc.dram_tensor("ident", [128, 128], BF, kind="ExternalInput")
    idf_d = nc.dram_tensor("identf", [97, 1], BF, kind="ExternalInput")
    if use_g2:
        g2_d = nc.dram_tensor("g2", [128, HPC * D], BF, kind="ExternalInput")
    out_d = nc.dram_tensor("out", [N1, C_S], F32, kind="ExternalOutput")

    with TileContext(nc) as tc:
        with (
            tc.tile_pool(name="const", bufs=1) as cpool,
            tc.tile_pool(name="work", bufs=4) as work,
            tc.tile_pool(name="norm", bufs=6) as npool,
            tc.tile_pool(name="expp", bufs=6) as expp,
            tc.tile_pool(name="psR", bufs=3, space="PSUM") as psR,
            tc.tile_pool(name="psO", bufs=1, space="PSUM") as psO,
            tc.tile_pool(name="psM", bufs=1, space="PSUM") as psM,
        ):
            # ---- constants / staging ----
            ident = cpool.tile([128, 128], BF)
            nc.sync.dma_start(ident[:, :], id_d.ap())
            identf = cpool.tile([97, 1], BF, tag="identf")
            nc.sync.dma_start(identf[:, :], idf_d.ap())

            wq_sb = cpool.tile([128, HPC * D], BF, tag="wq")
            wq_sb2 = cpool.tile([128, HPC * D], BF, tag="wq2")
            nc.sync.dma_start(wq_sb[:, :], wq_d.ap()[0:128, :])
            nc.sync.dma_start(wq_sb2[:, :], wq_d.ap()[128:256, :])
            wkv_sb = cpool.tile([128, HPC * 2 * D], BF, tag="wkv")
            wkv_sb2 = cpool.tile([128, HPC * 2 * D], BF, tag="wkv2")
            nc.sync.dma_start(wkv_sb[:, :], wkv_d.ap()[0:128, :])
            nc.sync.dma_start(wkv_sb2[:, :], wkv_d.ap()[128:256, :])
            wout_sb = cpool.tile([97, C_S], BF, tag="wout")
            nc.sync.dma_start(wout_sb[:, :], wout_d.ap())
            if use_g2:
                g2_sb = cpool.tile([128, HPC * D], BF, tag="g2")
                nc.sync.dma_start(g2_sb[:, :], g2_d.ap())

            s1T = [cpool.tile([128, N1], BF, tag=f"s1T{i}", name=f"s1T{i}") for i in range(2)]
            s2T = [cpool.tile([128, N2], BF, tag=f"s2T{i}", name=f"s2T{i}") for i in range(2)]
            # kv prologue consumes s2T first: stage it ahead of s1T
            for j in range(8):
                sl = slice(j * (N1 // 8), (j + 1) * (N1 // 8))
                for i in range(2):
                    nc.sync.dma_start(s2T[i][:, sl], s2T_d.ap()[i * 128:(i + 1) * 128, sl])
            for j in range(8):
                sl = slice(j * (N1 // 8), (j + 1) * (N1 // 8))
                for i in range(2):
                    nc.sync.dma_start(s1T[i][:, sl], s1T_d.ap()[i * 128:(i + 1) * 128, sl])

            # two-head packed transposed q/k: head0 dims in rows 0-31 with
            # mask row 32, head1 dims in rows 64-95 with mask row 96.  Rows
            # 33-63 / 97-127 are never streamed (K=33 APs).
            kT2 = cpool.tile([128, N2], BF, tag="kT2", name="kT2")
            qT2 = cpool.tile([128, N1], BF, tag="qT2", name="qT2")
            nc.sync.dma_start(kT2[32:33, :], km_d.ap())
            nc.sync.dma_start(kT2[96:97, :], km_d.ap())
            nc.sync.dma_start(qT2[32:33, :], qm_d.ap())
            nc.sync.dma_start(qT2[96:97, :], qm_d.ap())

            # v-extended: per kchunk 66 cols, 2 col-tiles of M=33:
            # [ones|v_h0 (32)][ones|v_h1 (32)]
            # ones lead so Z lands at oT partitions 0/64 (aligned).
            vx = cpool.tile([128, KCH * 66], BF, tag="vx")
            vx4 = vx[:, :].rearrange("p (k h x) -> p k h x", h=2, x=33)
            nc.vector.memset(vx4[:, :, :, 0:1], 1.0)

            oT_sb = cpool.tile([128, N1], BF, tag="oT")
            nc.vector.memset(oT_sb[:, :], 0.0)
            rzp = cpool.tile([128, QCH * HPC], F32, tag="rzp")

            # ---- phases A/B: projections + rms-norm + transposes ----
            NCHUNK = KCH + QCH  # 48
            kcp_all = cpool.tile([128, NCHUNK * HPC * D], BF, tag="kcp_all")
            ss_all = cpool.tile([128, NCHUNK * HPC], F32, tag="ss_all")

            def pass1_g(kcs, sT, w1, w2, kvside):
                # 4 chunks per PSUM bank: one kcp copy / sq / reduce / v-copy
                ng = len(kcs)
                ncol = w1.shape[1]
                ci0 = kcs[0] if kvside else KCH + kcs[0]
                pp = psM.tile([128, 4 * ncol], F32, tag="mx", name=f"pp{ci0}")
                for idx, kc in enumerate(kcs):
                    nc.tensor.matmul(pp[:, idx * ncol:(idx + 1) * ncol],
                                     sT[0][:, kc * 128:(kc + 1) * 128], w1[:, :],
                                     start=True, stop=False)
                    nc.tensor.matmul(pp[:, idx * ncol:(idx + 1) * ncol],
                                     sT[1][:, kc * 128:(kc + 1) * 128], w2[:, :],
                                     start=False, stop=True)
                kcp = kcp_all[:, ci0 * HPC * D:(ci0 + ng) * HPC * D]
                nc.vector.tensor_copy(
                    kcp.rearrange("p (c h d) -> p c h d", c=ng, d=D),
                    pp[:, 0:ng * ncol].rearrange("p (c h x) -> p c h x",
                                                 c=ng, h=HPC)[:, :, :, 0:D])
                sq = npool.tile([128, 4 * HPC * D], F32, tag="sq",
                                name=f"sq{ci0}")
                nc.gpsimd.tensor_tensor(sq[:, 0:ng * HPC * D], kcp, kcp,
                                        AluOpType.mult)
                nc.vector.reduce_sum(
                    ss_all[:, ci0 * HPC:(ci0 + ng) * HPC],
                    sq[:, 0:ng * HPC * D].rearrange("p (ch d) -> p ch d", d=D),
                    axis=AX.X)
                if kvside:  # copy v into vx cols 1-33 per head (+cast bf16)
                    nc.vector.tensor_copy(
                        vx[:, kcs[0] * 66:(kcs[0] + ng) * 66]
                        .rearrange("p (c h x) -> p c h x", c=ng, x=33)[:, :, :, 1:33],
                        pp[:, 0:ng * ncol].rearrange("p (c h x) -> p c h x",
                                                     c=ng, h=HPC)[:, :, :, D:2 * D])

            sr_all = cpool.tile([128, NCHUNK * HPC], F32, tag="sr_all")
            rinv_all = cpool.tile([128, NCHUNK * HPC], BF, tag="rinv_all")

            def rsqrt_batch(sl, bid):
                # rinv = 1/sqrt(ss/D + eps) entirely on DVE (bit-trick seed +
                # 2 Newton steps, ~5e-6 rel err).  Keeps Sqrt off ScalarE so
                # the in-order ACT stream carries nothing but Exp ops.
                w = sl.stop - sl.start
                x = sr_all[:, sl]
                nc.vector.tensor_scalar(x, ss_all[:, sl], 1.0 / D, EPS,
                                        AluOpType.mult, AluOpType.add)
                t = npool.tile([128, NCHUNK * HPC], I32, tag="nrt",
                               name=f"nrt{bid}")
                nc.vector.tensor_scalar(t[:, 0:w], x.bitcast(I32), 1, None,
                                        AluOpType.arith_shift_right)
                u = npool.tile([128, NCHUNK * HPC], I32, tag="nru",
                               name=f"nru{bid}")
                nc.vector.tensor_scalar(u[:, 0:w], t[:, 0:w], -1, 0x5F3759DF,
                                        AluOpType.mult, AluOpType.add)
                y = u[:, 0:w].bitcast(F32)
                for it in range(2):
                    a = npool.tile([128, NCHUNK * HPC], F32, tag="nra",
                                   name=f"nra{bid}_{it}")
                    nc.vector.tensor_tensor(a[:, 0:w], y, y, AluOpType.mult)
                    b = npool.tile([128, NCHUNK * HPC], F32, tag="nrb",
                                   name=f"nrb{bid}_{it}")
                    nc.vector.tensor_tensor(b[:, 0:w], a[:, 0:w], x,
                                            AluOpType.mult)
                    c = npool.tile([128, NCHUNK * HPC], F32, tag="nrc",
                                   name=f"nrc{bid}_{it}")
                    nc.vector.tensor_scalar(c[:, 0:w], b[:, 0:w], -0.5, 1.5,
                                            AluOpType.mult, AluOpType.add)
                    if it == 0:
                        dst = npool.tile([128, NCHUNK * HPC], F32, tag="nry",
                                         name=f"nry{bid}_{it}")
                        out = dst[:, 0:w]
                    else:
                        out = rinv_all[:, sl]  # bf16 writeback on last step
                    nc.vector.tensor_tensor(out, y, c[:, 0:w], AluOpType.mult)
                    y = out

            def pass2(ci, kc, dstT, qside):
                kcp = kcp_all[:, ci * HPC * D:(ci + 1) * HPC * D]
                pre = npool.tile([128, HPC * D], BF, tag="pre", name=f"pre{ci}")
                rb = rinv_all[:, ci * HPC:(ci + 1) * HPC]
                nc.gpsimd.tensor_tensor(
                    pre[:, :].rearrange("p (h d) -> p h d", d=D),
                    kcp.rearrange("p (h d) -> p h d", d=D),
                    rb[:, :, None].broadcast_to([128, HPC, D]),
                    AluOpType.mult)
                if use_g2 and qside:  # q side carries the gq*gk factor
                    nc.vector.tensor_tensor(pre[:, :], pre[:, :], g2_sb[:, :],
                                            AluOpType.mult)
                tp = psM.tile([HPC * D, 128], BF, tag="mx", name=f"tp{ci}")
                nc.tensor.transpose(tp[:, :], pre[:, :], ident[:, :])
                nc.vector.tensor_copy(dstT[0:32, kc * 128:(kc + 1) * 128],
                                      tp[0:32, :])
                nc.vector.tensor_copy(dstT[64:96, kc * 128:(kc + 1) * 128],
                                      tp[32:64, :])

            def norm_batch(chunks, kvside):
                for g0 in range(0, len(chunks), 4):
                    kcs = chunks[g0:g0 + 4]
                    if kvside:
                        pass1_g(kcs, s2T, wkv_sb, wkv_sb2, True)
                    else:
                        pass1_g(kcs, s1T, wq_sb, wq_sb2, False)
                ci0 = (chunks[0] if kvside else KCH + chunks[0]) * HPC
                ci1 = (chunks[-1] if kvside else KCH + chunks[-1]) * HPC + HPC
                rsqrt_batch(slice(ci0, ci1), f"b{ci0}")
                for kc in chunks:
                    if kvside:
                        pass2(kc, kc, kT2, False)
                    else:
                        pass2(KCH + kc, kc, qT2, True)

            oT_tiles = {}
            pending_drain = []

            def drain(qb):
                # 1/Z from the Z rows already sitting in oT_sb (parts 0/64)
                ztr = psM.tile([128, 8 * HPC], BF, tag="mx", name=f"ztr{qb}")
                for c in range(4):
                    for h in range(HPC):
                        i = c * HPC + h
                        nc.tensor.transpose(
                            ztr[:, 2 * i:2 * i + 1],
                            oT_sb[h * 64:h * 64 + 1,
                                  qb * QB + c * 128:qb * QB + (c + 1) * 128],
                            identf[h * 64:h * 64 + 1, 0:1])
                nc.vector.reciprocal(
                    rzp[:, qb * 4 * HPC:(qb + 1) * 4 * HPC],
                    ztr[:, :].rearrange("p (i two) -> p i two", two=2)[:, :, 0])

            def attend(qb, fillers=(), pattern=PAT_MAIN):
                fillers = list(fillers)
                fillers[1:1] = pending_drain
                del pending_drain[:]
                nf = len(fillers)
                qsl = slice(qb * QB, (qb + 1) * QB)
                oT = None
                for kc in range(KCH):
                    while fillers and (nf - len(fillers)) * KCH <= kc * nf:
                        fillers.pop(0)()
                    if oT is None:
                        oT = psO.tile([128, QB], F32, tag="oT", name=f"oT_{qb}")
                        oT_tiles[qb] = oT
                    sc = psR.tile([128, HPC * QB], F32, tag="sc",
                                  name=f"sc_{qb}_{kc}")
                    nc.tensor.matmul(
                        sc[:, 0:QB],
                        kT2[0:33, kc * 128:(kc + 1) * 128],
                        qT2[0:33, qsl], start=True, stop=True)
                    nc.tensor.matmul(
                        sc[:, QB:2 * QB],
                        kT2[64:97, kc * 128:(kc + 1) * 128],
                        qT2[64:97, qsl], start=True, stop=True)
                    ex = expp.tile([128, HPC * QB], BF, tag="ex",
                                   name=f"ex_{qb}_{kc}")
                    eng = pattern[kc % len(pattern)]
                    if eng == "A":
                        nc.scalar.activation(ex[:, :], sc[:, :], AF.Exp,
                                             scale=SCALE)
                    elif eng == "D":
                        nc.vector.tensor_scalar(
                            ex[:, :].bitcast(I16), sc[:, :], AS_EXP, BS_EXP,
                            AluOpType.mult, AluOpType.add)
                    else:
                        nc.gpsimd.tensor_scalar(
                            ex[:, :].bitcast(I16), sc[:, :], AS_EXP, BS_EXP,
                            AluOpType.mult, AluOpType.add)
                    for j in range(2):
                        nc.tensor.matmul(
                            oT[j * 64:j * 64 + 33, :],
                            vx[:, kc * 66 + j * 33:kc * 66 + (j + 1) * 33],
                            ex[:, j * QB:(j + 1) * QB],
                            start=(kc == 0), stop=(kc == KCH - 1),
                            tile_position=(0, j * 64))
                nc.vector.tensor_copy(oT_sb[0:33, qsl], oT[0:33, :])
                nc.vector.tensor_copy(oT_sb[64:97, qsl], oT[64:97, :])
                oT_tiles.pop(qb, None)
                if os.environ.get("KDEFER", "1") == "1":
                    pending_drain.append(lambda: drain(qb))
                else:
                    drain(qb)

            def proj_out(qc):
                osl = slice(qc * 128, (qc + 1) * 128)
                op0 = psM.tile([128, C_S], F32, tag="mx", name=f"op0_{qc}")
                nc.tensor.matmul(op0[:, :], oT_sb[0:33, osl], wout_sb[0:33, :],
                                 start=True, stop=True)
                op1 = psM.tile([128, C_S], F32, tag="mx", name=f"op1_{qc}")
                nc.tensor.matmul(op1[:, :], oT_sb[64:97, osl],
                                 wout_sb[64:97, :], start=True, stop=True)
                t0 = work.tile([128, C_S], F32, tag="t0", name=f"t0_{qc}")
                nc.vector.tensor_scalar(t0[:, :], op0[:, :],
                                        rzp[:, qc * HPC:qc * HPC + 1], None,
                                        AluOpType.mult)
                ops = work.tile([128, C_S], F32, tag="osb", name=f"osb_{qc}")
                nc.vector.scalar_tensor_tensor(
                    ops[:, :], op1[:, :], rzp[:, qc * HPC + 1:qc * HPC + 2],
                    t0[:, :], AluOpType.mult, AluOpType.add)
                nc.sync.dma_start(out_d.ap()[osl, :], ops[:, :])

            # prologue schedule: attend(0) needs qT2 chunks 0-3 and kT2
            # incrementally; emit the minimum before it and overlap the rest.
            # The out-projection for q-block qb is emitted after attend(qb+1)
            # so its matmuls fill PE bubbles instead of forming a tail.
            norm_batch(list(range(0, 8)), kvside=False)
            norm_batch(list(range(0, 8)), kvside=True)
            norm_batch(list(range(8, 16)), kvside=True)
            norm_batch(list(range(16, 24)), kvside=True)
            if KBISECT >= 2:
                f0 = [(lambda g0=g0: pass1_g(list(range(g0, g0 + 2)),
                                             s1T, wq_sb, wq_sb2, False))
                      for g0 in range(8, 24, 2)]
                attend(0, f0, PAT_EARLY)
                q_sl2 = slice((KCH + 8) * HPC, (KCH + 24) * HPC)
                rsqrt_batch(q_sl2, "q2")
            if KBISECT >= 3:
                f1 = [(lambda kc=kc: pass2(KCH + kc, kc, qT2, True))
                      for kc in range(8, 24)]
                attend(1, f1, PAT_EARLY)
            if KBISECT >= 4:
                for qb in range(2, NQB):
                    hi = (qb - 1) * 4 + (4 if qb == NQB - 1 else 0)
                    fd = ([(lambda qc=qc: proj_out(qc))
                           for qc in range((qb - 2) * 4, hi)]
                          if KBISECT >= 5 else [])
                    attend(qb, fd, PAT_MAIN)
            if KBISECT >= 5:
                for d in pending_drain:
                    d()
                del pending_drain[:]
                for qc in range((NQB - 1) * 4, NQB * 4):
                    proj_out(qc)
            else:
                zo = work.tile([128, C_S], F32, tag="osb")
                nc.vector.memset(zo[:, :], 0.0)
                for qc in range(QCH):
                    nc.sync.dma_start(out_d.ap()[qc * 128:(qc + 1) * 128, :],
                                      zo[:, :])

    nc.compile()
    return nc


def _pad_wout(w):
    # match oT_sb partition layout: [Z | h0 d (32) | ... | Z | h1 d (32)]
    wp = np.zeros((97, C_S), np.float32)
    wp[1:33] = w[0:32]
    wp[65:97] = w[32:64]
    return wp.astype(bf16)


def _host_prep(inputs):
    s1 = np.asarray(inputs["s1"], np.float32)
    s2 = np.asarray(inputs["s2"], np.float32)
    ridx1 = np.asarray(inputs["ridx1"], np.int32)
    ct1 = np.asarray(inputs["ct1"], np.int32)
    mask1 = np.asarray(inputs["mask1"], np.int32)
    mask2 = np.asarray(inputs["mask2"], np.int32)
    Wq = np.asarray(inputs["Wq"], np.float32)
    Wkv = np.asarray(inputs["Wkv"], np.float32)
    Wout = np.asarray(inputs["Wout"], np.float32)
    gq = np.asarray(inputs["gq"], np.float32)
    gk = np.asarray(inputs["gk"], np.float32)

    ct_idx = np.take_along_axis(ridx1, ct1[:, None], axis=1)
    pos = (ridx1 - ct_idx).astype(np.float32)
    half = C_S // 2
    freqs = np.exp(-np.log(10000.0) * np.arange(half, dtype=np.float32) / half)
    ang = pos[..., None] * freqs
    s1e = s1 + np.concatenate([np.sin(ang), np.cos(ang)], axis=-1).astype(np.float32)

    m1 = mask1.astype(np.float32)
    km = (mask2.astype(np.float32) - 1.0) * MASKV / SCALE

    g2 = gq * gk
    use_g2 = not np.allclose(g2, 1.0)

    ident = np.eye(128, dtype=bf16)
    in_maps = []
    for c in range(NCORES):
        b, hp = c // 4, c % 4
        m = {
            "s1T": np.ascontiguousarray(s1e[b].T).astype(bf16),
            "s2T": np.ascontiguousarray(s2[b].T).astype(bf16),
            "wq": np.ascontiguousarray(Wq[:, hp * HPC * D:(hp + 1) * HPC * D]).astype(bf16),
            "wkv": np.ascontiguousarray(Wkv[:, hp * HPC * 2 * D:(hp + 1) * HPC * 2 * D]).astype(bf16),
            "wout": _pad_wout(Wout[hp * HPC * D:(hp + 1) * HPC * D, :]),
            "qm": m1[b][None, :].astype(bf16),
            "km": km[b][None, :].astype(bf16),
            "ident": ident,
            "identf": np.ones((97, 1), dtype=bf16),
        }
        if use_g2:
            m["g2"] = np.tile(g2[None, hp * HPC * D:(hp + 1) * HPC * D], (128, 1)).astype(bf16)
        in_maps.append(m)
    return in_maps, use_g2, np.asarray(inputs["b_out"], np.float32)


def _run(inputs, trace=False, **kw):
    in_maps, use_g2, b_out = _host_prep(inputs)
    key = ("nc", use_g2)
    if key not in _cache:
        _cache[key] = _build(use_g2)
    nc = _cache[key]
    res = bass_utils.run_bass_kernel_spmd(
        nc, in_maps, core_ids=list(range(NCORES)), trace=trace, **kw)
    out = np.zeros((B, N1, C_S), np.float32)
    for c in range(NCORES):
        out[c // 4] += res.results[c]["out"]
    out += b_out[None, None, :]
    return out, res


def kernel(**inputs) -> np.ndarray:
    out, _ = _run(inputs, trace=False)
    return out


# revision 39
# speedup vs baseline: 1.0440x; 1.0440x over previous
"""Trainium2 Bass kernel for nn_Attention_12146167513140.

Distributed dense attention over 8 NeuronCores.

Sharding: core c in 0..7 -> (b = c//4, head-pair hp = c%4).  Each core
computes the full [3072 q x 3072 k] attention for its 2 heads of its
batch, producing a partial output projection [3072, 256]; the host sums
the 4 partials per batch and adds b_out.

Device pipeline per core (all matmuls bf16, accumulation f32 in PSUM):
  A) kv = s2 @ Wkv_pair -> rms-norm k -> kT2 (PE transpose),
     v -> vx tiles (4 sub-tiles of 17 cols: 16 v dims + ones col for Z)
  B) q = s1e @ Wq_pair -> rms-norm q -> qT2
  C) flash-style, PE-array tiled:
     - QK: 2-way row tiling (K=33: 32 dims + mask row).  Head 0 in array
       rows 0-63, head 1 in rows 64-127, concurrent.
     - exp: dispatched per key-chunk to ScalarE (exact exp, scale fused)
       or DVE via a 1-op Schraudolph fast-exp (f32 mult-add with int16
       writeback; the int16 bit pattern IS the bf16 exp approximation;
       RNE + saturation verified on HW).  Pattern "ADA" (period 3) pins
       one engine per sc-ring slot.  Mask bias is -30/SCALE so fast-exp
       inputs stay in the int16-safe range.
     - PV: 2-way column tiling (M=33 = ones|v per head); the leading
       ones column accumulates the softmax denominator Z at aligned
       PSUM partitions 0/64.
  D) 1/Z via tiny PE transposes of the Z rows already in oT_sb + DVE
     reciprocal; out_partial = (oT/Z).T @ Wout (zero-padded rows drop
     the Z row and match the oT_sb layout), 2-way row-tiled.

Host-side prep: sinusoidal positional embedding (index arithmetic),
transposes, bf16 casts, mask row encoding.
"""

import contextlib
import ctypes
import sys
import types

import numpy as np
import ml_dtypes

import concourse.bacc as bacc
import concourse.mybir as mybir
from concourse import bass_utils
from concourse.tile import TileContext
from concourse.alu_op_type import AluOpType
from concourse.mybir import ActivationFunctionType as AF


def _ensure_trace_support():
    """The container's antenv package lacks axon_hooks; bass_utils
    imports it when tracing is requested (e.g. via BASS_TRACE).  Install
    a functional shim so a traced run works instead of crashing, and
    make the artifact upload a no-op (no bucket access here)."""
    try:
        import antenv.axon_hooks  # noqa: F401
        return
    except ImportError:
        pass
    mod = types.ModuleType("antenv.axon_hooks")
    mod._hook = None
    mod.set_axon_ntff_profile_hook = lambda h: setattr(mod, "_hook", h)
    mod.get_axon_ntff_profile_hook = lambda: mod._hook
    try:
        import antenv
        sys.modules["antenv.axon_hooks"] = mod
        antenv.axon_hooks = mod
    except ImportError:
        sys.modules["antenv.axon_hooks"] = mod

    def _ntff_hook(so_path):
        try:
            lib = ctypes.CDLL(so_path)
        except OSError:
            return None
        if not hasattr(lib, "axon_start_nrt_profile"):
            return None
        lib.axon_start_nrt_profile.argtypes = [ctypes.POINTER(ctypes.c_int64),
                                               ctypes.c_size_t]
        lib.axon_start_nrt_profile.restype = ctypes.c_int64
        lib.axon_stop_nrt_profile.argtypes = [ctypes.c_char_p]
        lib.axon_stop_nrt_profile.restype = ctypes.c_int64

        @contextlib.contextmanager
        def _hook(output_dir, device_ids):
            import jax
            jax.devices()
            if device_ids:
                ids = (ctypes.c_int64 * len(device_ids))(*device_ids)
                rc = lib.axon_start_nrt_profile(ids, len(device_ids))
            else:
                rc = lib.axon_start_nrt_profile(None, 0)
            if rc != 0:
                raise RuntimeError(f"axon_start_nrt_profile rc={rc}")
            try:
                yield
            finally:
                lib.axon_stop_nrt_profile(str(output_dir).encode())

        return _hook

    mod.set_axon_ntff_profile_hook(_ntff_hook("/opt/axon/libaxon_pjrt.so"))

    _orig_upload = bass_utils.upload_artifacts

    def _safe_upload(tmpdir):
        try:
            return _orig_upload(tmpdir)
        except Exception:
            return tmpdir

    bass_utils.upload_artifacts = _safe_upload


_ensure_trace_support()

AX = mybir.AxisListType
I16 = mybir.dt.int16
I32 = mybir.dt.int32
BF = mybir.dt.bfloat16
F32 = mybir.dt.float32
bf16 = ml_dtypes.bfloat16

B, N1, N2 = 2, 3072, 3072
C_S, H, D = 256, 8, 32
INF = 100000.0
EPS = 1e-8
SCALE = float(np.sqrt(1.0 / (3 * D)))
MASKV = 30.0          # effective -30 in the exp argument for masked keys

NCORES = 8
HPC = 2            # heads per core
KCH = N2 // 128    # 24 key chunks
QCH = N1 // 128    # 24 q row chunks
QB = 512           # q block for scores free dim
NQB = N1 // QB     # 6
VJ = 17            # PV col-tile width: 16 v dims + ones col

# Schraudolph fast-exp constants: int16(y*AS + BS) viewed as bf16 ~ exp(y)
# for y = raw score (SCALE folded into AS).  Calibrated on HW (RNE writeback).
LN2 = float(np.log(2.0))
AS_EXP = SCALE * 128.0 / LN2
BS_EXP = 127.0 * 128.0 - 7.4

import os
# exp engine dispatch patterns (per key chunk index):
# A = ScalarE exact exp, D = DVE fast-exp (GPSIMD cannot read PSUM)
PAT_EARLY = os.environ.get("KPE", "A")    # qb 0-1: ScalarE only; DVE runs prologue fillers
PAT_MAIN = os.environ.get("KPM", "ADA")   # qb 2+: period 3 = one engine per ring slot

_cache = {}
KBISECT = int(os.environ.get("KBISECT", "5"))


def _build(use_g2: bool):
    nc = bacc.Bacc("TRN2", target_bir_lowering=False, debug=False, num_devices=NCORES)

    s1T_d = nc.dram_tensor("s1T", [C_S, N1], BF, kind="ExternalInput")
    s2T_d = nc.dram_tensor("s2T", [C_S, N2], BF, kind="ExternalInput")
    wq_d = nc.dram_tensor("wq", [C_S, HPC * D], BF, kind="ExternalInput")
    wkv_d = nc.dram_tensor("wkv", [C_S, HPC * 2 * D], BF, kind="ExternalInput")
    wout_d = nc.dram_tensor("wout", [97, C_S], BF, kind="ExternalInput")
    qm_d = nc.dram_tensor("qm", [1, N1], BF, kind="ExternalInput")
    km_d = nc.dram_tensor("km", [1, N2], BF, kind="ExternalInput")
    id_d = nc.dram_tensor("ident", [128, 128], BF, kind="ExternalInput")
    idf_d = nc.dram_tensor("identf", [97, 1], BF, kind="ExternalInput")
    if use_g2:
        g2_d = nc.dram_tensor("g2", [128, HPC * D], BF, kind="ExternalInput")
    out_d = nc.dram_tensor("out", [N1, C_S], F32, kind="ExternalOutput")

    with TileContext(nc) as tc:
        with (
            tc.tile_pool(name="const", bufs=1) as cpool,
            tc.tile_pool(name="work", bufs=4) as work,
            tc.tile_pool(name="norm", bufs=6) as npool,
            tc.tile_pool(name="expp", bufs=6) as expp,
            tc.tile_pool(name="psR", bufs=3, space="PSUM") as psR,
            tc.tile_pool(name="psO", bufs=1, space="PSUM") as psO,
            tc.tile_pool(name="psM", bufs=1, space="PSUM") as psM,
        ):
            # ---- constants / staging ----
            ident = cpool.tile([128, 128], BF)
            nc.sync.dma_start(ident[:, :], id_d.ap())
            identf = cpool.tile([97, 1], BF, tag="identf")
            nc.sync.dma_start(identf[:, :], idf_d.ap())

            wq_sb = cpool.tile([128, HPC * D], BF, tag="wq")
            wq_sb2 = cpool.tile([128, HPC * D], BF, tag="wq2")
            nc.sync.dma_start(wq_sb[:, :], wq_d.ap()[0:128, :])
            nc.sync.dma_start(wq_sb2[:, :], wq_d.ap()[128:256, :])
            wkv_sb = cpool.tile([128, HPC * 2 * D], BF, tag="wkv")
            wkv_sb2 = cpool.tile([128, HPC * 2 * D], BF, tag="wkv2")
            nc.sync.dma_start(wkv_sb[:, :], wkv_d.ap()[0:128, :])
            nc.sync.dma_start(wkv_sb2[:, :], wkv_d.ap()[128:256, :])
            wout_sb = cpool.tile([97, C_S], BF, tag="wout")
            nc.sync.dma_start(wout_sb[:, :], wout_d.ap())
            if use_g2:
                g2_sb = cpool.tile([128, HPC * D], BF, tag="g2")
                nc.sync.dma_start(g2_sb[:, :], g2_d.ap())

            s1T = [cpool.tile([128, N1], BF, tag=f"s1T{i}", name=f"s1T{i}") for i in range(2)]
            s2T = [cpool.tile([128, N2], BF, tag=f"s2T{i}", name=f"s2T{i}") for i in range(2)]
            for j in range(8):
                sl = slice(j * (N1 // 8), (j + 1) * (N1 // 8))
                for i in range(2):
                    nc.sync.dma_start(s1T[i][:, sl], s1T_d.ap()[i * 128:(i + 1) * 128, sl])
                    nc.sync.dma_start(s2T[i][:, sl], s2T_d.ap()[i * 128:(i + 1) * 128, sl])

            # two-head packed transposed q/k: head0 dims in rows 0-31 with
            # mask row 32, head1 dims in rows 64-95 with mask row 96.  Rows
            # 33-63 / 97-127 are never streamed (K=33 APs).
            kT2 = cpool.tile([128, N2], BF, tag="kT2", name="kT2")
            qT2 = cpool.tile([128, N1], BF, tag="qT2", name="qT2")
            nc.sync.dma_start(kT2[32:33, :], km_d.ap())
            nc.sync.dma_start(kT2[96:97, :], km_d.ap())
            nc.sync.dma_start(qT2[32:33, :], qm_d.ap())
            nc.sync.dma_start(qT2[96:97, :], qm_d.ap())

            # v-extended: per kchunk 66 cols, 2 col-tiles of M=33:
            # [ones|v_h0 (32)][ones|v_h1 (32)]
            # ones lead so Z lands at oT partitions 0/64 (aligned).
            vx = cpool.tile([128, KCH * 66], BF, tag="vx")
            vx4 = vx[:, :].rearrange("p (k h x) -> p k h x", h=2, x=33)
            nc.vector.memset(vx4[:, :, :, 0:1], 1.0)

            oT_sb = cpool.tile([128, N1], BF, tag="oT")
            nc.vector.memset(oT_sb[:, :], 0.0)
            rzp = cpool.tile([128, QCH * HPC], F32, tag="rzp")

            # ---- phases A/B: projections + rms-norm + transposes ----
            NCHUNK = KCH + QCH  # 48
            kcp_all = cpool.tile([128, NCHUNK * HPC * D], BF, tag="kcp_all")
            ss_all = cpool.tile([128, NCHUNK * HPC], F32, tag="ss_all")

            def pass1_g(kcs, sT, w1, w2, kvside):
                # 4 chunks per PSUM bank: one kcp copy / sq / reduce / v-copy
                ng = len(kcs)
                ncol = w1.shape[1]
                ci0 = kcs[0] if kvside else KCH + kcs[0]
                pp = psM.tile([128, 4 * ncol], F32, tag="mx", name=f"pp{ci0}")
                for idx, kc in enumerate(kcs):
                    nc.tensor.matmul(pp[:, idx * ncol:(idx + 1) * ncol],
                                     sT[0][:, kc * 128:(kc + 1) * 128], w1[:, :],
                                     start=True, stop=False)
                    nc.tensor.matmul(pp[:, idx * ncol:(idx + 1) * ncol],
                                     sT[1][:, kc * 128:(kc + 1) * 128], w2[:, :],
                                     start=False, stop=True)
                kcp = kcp_all[:, ci0 * HPC * D:(ci0 + ng) * HPC * D]
                nc.vector.tensor_copy(
                    kcp.rearrange("p (c h d) -> p c h d", c=ng, d=D),
                    pp[:, 0:ng * ncol].rearrange("p (c h x) -> p c h x",
                                                 c=ng, h=HPC)[:, :, :, 0:D])
                sq = npool.tile([128, 4 * HPC * D], F32, tag="sq",
                                name=f"sq{ci0}")
                nc.gpsimd.tensor_tensor(sq[:, 0:ng * HPC * D], kcp, kcp,
                                        AluOpType.mult)
                nc.vector.reduce_sum(
                    ss_all[:, ci0 * HPC:(ci0 + ng) * HPC],
                    sq[:, 0:ng * HPC * D].rearrange("p (ch d) -> p ch d", d=D),
                    axis=AX.X)
                if kvside:  # copy v into vx cols 1-33 per head (+cast bf16)
                    nc.vector.tensor_copy(
                        vx[:, kcs[0] * 66:(kcs[0] + ng) * 66]
                        .rearrange("p (c h x) -> p c h x", c=ng, x=33)[:, :, :, 1:33],
                        pp[:, 0:ng * ncol].rearrange("p (c h x) -> p c h x",
                                                     c=ng, h=HPC)[:, :, :, D:2 * D])

            sr_all = cpool.tile([128, NCHUNK * HPC], F32, tag="sr_all")
            rinv_all = cpool.tile([128, NCHUNK * HPC], BF, tag="rinv_all")

            def rsqrt_batch(sl, bid):
                # rinv = 1/sqrt(ss/D + eps) entirely on DVE (bit-trick seed +
                # 2 Newton steps, ~5e-6 rel err).  Keeps Sqrt off ScalarE so
                # the in-order ACT stream carries nothing but Exp ops.
                w = sl.stop - sl.start
                x = sr_all[:, sl]
                nc.vector.tensor_scalar(x, ss_all[:, sl], 1.0 / D, EPS,
                                        AluOpType.mult, AluOpType.add)
                t = npool.tile([128, NCHUNK * HPC], I32, tag="nrt",
                               name=f"nrt{bid}")
                nc.vector.tensor_scalar(t[:, 0:w], x.bitcast(I32), 1, None,
                                        AluOpType.arith_shift_right)
                u = npool.tile([128, NCHUNK * HPC], I32, tag="nru",
                               name=f"nru{bid}")
                nc.vector.tensor_scalar(u[:, 0:w], t[:, 0:w], -1, 0x5F3759DF,
                                        AluOpType.mult, AluOpType.add)
                y = u[:, 0:w].bitcast(F32)
                for it in range(2):
                    a = npool.tile([128, NCHUNK * HPC], F32, tag="nra",
                                   name=f"nra{bid}_{it}")
                    nc.vector.tensor_tensor(a[:, 0:w], y, y, AluOpType.mult)
                    b = npool.tile([128, NCHUNK * HPC], F32, tag="nrb",
                                   name=f"nrb{bid}_{it}")
                    nc.vector.tensor_tensor(b[:, 0:w], a[:, 0:w], x,
                                            AluOpType.mult)
                    c = npool.tile([128, NCHUNK * HPC], F32, tag="nrc",
                                   name=f"nrc{bid}_{it}")
                    nc.vector.tensor_scalar(c[:, 0:w], b[:, 0:w], -0.5, 1.5,
                                            AluOpType.mult, AluOpType.add)
                    if it == 0:
                        dst = npool.tile([128, NCHUNK * HPC], F32, tag="nry",
                                         name=f"nry{bid}_{it}")
                        out = dst[:, 0:w]
                    else:
                        out = rinv_all[:, sl]  # bf16 writeback on last step
                    nc.vector.tensor_tensor(out, y, c[:, 0:w], AluOpType.mult)
                    y = out

            def pass2(ci, kc, dstT, qside):
                kcp = kcp_all[:, ci * HPC * D:(ci + 1) * HPC * D]
                pre = npool.tile([128, HPC * D], BF, tag="pre", name=f"pre{ci}")
                rb = rinv_all[:, ci * HPC:(ci + 1) * HPC]
                nc.gpsimd.tensor_tensor(
                    pre[:, :].rearrange("p (h d) -> p h d", d=D),
                    kcp.rearrange("p (h d) -> p h d", d=D),
                    rb[:, :, None].broadcast_to([128, HPC, D]),
                    AluOpType.mult)
                if use_g2 and qside:  # q side carries the gq*gk factor
                    nc.vector.tensor_tensor(pre[:, :], pre[:, :], g2_sb[:, :],
                                            AluOpType.mult)
                tp = psM.tile([HPC * D, 128], BF, tag="mx", name=f"tp{ci}")
                nc.tensor.transpose(tp[:, :], pre[:, :], ident[:, :])
                nc.vector.tensor_copy(dstT[0:32, kc * 128:(kc + 1) * 128],
                                      tp[0:32, :])
                nc.vector.tensor_copy(dstT[64:96, kc * 128:(kc + 1) * 128],
                                      tp[32:64, :])

            def norm_batch(chunks, kvside):
                for g0 in range(0, len(chunks), 4):
                    kcs = chunks[g0:g0 + 4]
                    if kvside:
                        pass1_g(kcs, s2T, wkv_sb, wkv_sb2, True)
                    else:
                        pass1_g(kcs, s1T, wq_sb, wq_sb2, False)
                ci0 = (chunks[0] if kvside else KCH + chunks[0]) * HPC
                ci1 = (chunks[-1] if kvside else KCH + chunks[-1]) * HPC + HPC
                rsqrt_batch(slice(ci0, ci1), f"b{ci0}")
                for kc in chunks:
                    if kvside:
                        pass2(kc, kc, kT2, False)
                    else:
                        pass2(KCH + kc, kc, qT2, True)

            oT_tiles = {}
            pending_drain = []

            def drain(qb):
                # 1/Z from the Z rows already sitting in oT_sb (parts 0/64)
                ztr = psM.tile([128, 8 * HPC], BF, tag="mx", name=f"ztr{qb}")
                for c in range(4):
                    for h in range(HPC):
                        i = c * HPC + h
                        nc.tensor.transpose(
                            ztr[:, 2 * i:2 * i + 1],
                            oT_sb[h * 64:h * 64 + 1,
                                  qb * QB + c * 128:qb * QB + (c + 1) * 128],
                            identf[h * 64:h * 64 + 1, 0:1])
                nc.vector.reciprocal(
                    rzp[:, qb * 4 * HPC:(qb + 1) * 4 * HPC],
                    ztr[:, :].rearrange("p (i two) -> p i two", two=2)[:, :, 0])

            def attend(qb, fillers=(), pattern=PAT_MAIN):
                fillers = list(fillers)
                fillers[1:1] = pending_drain
                del pending_drain[:]
                nf = len(fillers)
                qsl = slice(qb * QB, (qb + 1) * QB)
                oT = None
                for kc in range(KCH):
                    while fillers and (nf - len(fillers)) * KCH <= kc * nf:
                        fillers.pop(0)()
                    if oT is None:
                        oT = psO.tile([128, QB], F32, tag="oT", name=f"oT_{qb}")
                        oT_tiles[qb] = oT
                    sc = psR.tile([128, HPC * QB], F32, tag="sc",
                                  name=f"sc_{qb}_{kc}")
                    nc.tensor.matmul(
                        sc[:, 0:QB],
                        kT2[0:33, kc * 128:(kc + 1) * 128],
                        qT2[0:33, qsl], start=True, stop=True)
                    nc.tensor.matmul(
                        sc[:, QB:2 * QB],
                        kT2[64:97, kc * 128:(kc + 1) * 128],
                        qT2[64:97, qsl], start=True, stop=True)
                    ex = expp.tile([128, HPC * QB], BF, tag="ex",
                                   name=f"ex_{qb}_{kc}")
                    eng = pattern[kc % len(pattern)]
                    if eng == "A":
                        nc.scalar.activation(ex[:, :], sc[:, :], AF.Exp,
                                             scale=SCALE)
                    elif eng == "D":
                        nc.vector.tensor_scalar(
                            ex[:, :].bitcast(I16), sc[:, :], AS_EXP, BS_EXP,
                            AluOpType.mult, AluOpType.add)
                    else:
                        nc.gpsimd.tensor_scalar(
                            ex[:, :].bitcast(I16), sc[:, :], AS_EXP, BS_EXP,
                            AluOpType.mult, AluOpType.add)
                    for j in range(2):
                        nc.tensor.matmul(
                            oT[j * 64:j * 64 + 33, :],
                            vx[:, kc * 66 + j * 33:kc * 66 + (j + 1) * 33],
                            ex[:, j * QB:(j + 1) * QB],
                            start=(kc == 0), stop=(kc == KCH - 1),
                            tile_position=(0, j * 64))
                nc.vector.tensor_copy(oT_sb[0:33, qsl], oT[0:33, :])
                nc.vector.tensor_copy(oT_sb[64:97, qsl], oT[64:97, :])
                oT_tiles.pop(qb, None)
                if os.environ.get("KDEFER", "1") == "1":
                    pending_drain.append(lambda: drain(qb))
                else:
                    drain(qb)

            def proj_out(qc):
                osl = slice(qc * 128, (qc + 1) * 128)
                op0 = psM.tile([128, C_S], F32, tag="mx", name=f"op0_{qc}")
                nc.tensor.matmul(op0[:, :], oT_sb[0:33, osl], wout_sb[0:33, :],
                                 start=True, stop=True)
                op1 = psM.tile([128, C_S], F32, tag="mx", name=f"op1_{qc}")
                nc.tensor.matmul(op1[:, :], oT_sb[64:97, osl],
                                 wout_sb[64:97, :], start=True, stop=True)
                t0 = work.tile([128, C_S], F32, tag="t0", name=f"t0_{qc}")
                nc.vector.tensor_scalar(t0[:, :], op0[:, :],
                                        rzp[:, qc * HPC:qc * HPC + 1], None,
                                        AluOpType.mult)
                ops = work.tile([128, C_S], F32, tag="osb", name=f"osb_{qc}")
                nc.vector.scalar_tensor_tensor(
                    ops[:, :], op1[:, :], rzp[:, qc * HPC + 1:qc * HPC + 2],
                    t0[:, :], AluOpType.mult, AluOpType.add)
                nc.sync.dma_start(out_d.ap()[osl, :], ops[:, :])

            # prologue schedule: attend(0) needs qT2 chunks 0-3 and kT2
            # incrementally; emit the minimum before it and overlap the rest.
            # The out-projection for q-block qb is emitted after attend(qb+1)
            # so its matmuls fill PE bubbles instead of forming a tail.
            norm_batch(list(range(0, 8)), kvside=False)
            norm_batch(list(range(0, 8)), kvside=True)
            norm_batch(list(range(8, 16)), kvside=True)
            norm_batch(list(range(16, 24)), kvside=True)
            if KBISECT >= 2:
                f0 = [(lambda g0=g0: pass1_g(list(range(g0, g0 + 2)),
                                             s1T, wq_sb, wq_sb2, False))
                      for g0 in range(8, 24, 2)]
                attend(0, f0, PAT_EARLY)
                q_sl2 = slice((KCH + 8) * HPC, (KCH + 24) * HPC)
                rsqrt_batch(q_sl2, "q2")
            if KBISECT >= 3:
                f1 = [(lambda kc=kc: pass2(KCH + kc, kc, qT2, True))
                      for kc in range(8, 24)]
                attend(1, f1, PAT_EARLY)
            if KBISECT >= 4:
                for qb in range(2, NQB):
                    hi = (qb - 1) * 4 + (4 if qb == NQB - 1 else 0)
                    fd = ([(lambda qc=qc: proj_out(qc))
                           for qc in range((qb - 2) * 4, hi)]
                          if KBISECT >= 5 else [])
                    attend(qb, fd, PAT_MAIN)
            if KBISECT >= 5:
                for d in pending_drain:
                    d()
                del pending_drain[:]
                for qc in range((NQB - 1) * 4, NQB * 4):
                    proj_out(qc)
            else:
                zo = work.tile([128, C_S], F32, tag="osb")
                nc.vector.memset(zo[:, :], 0.0)
                for qc in range(QCH):
                    nc.sync.dma_start(out_d.ap()[qc * 128:(qc + 1) * 128, :],
                                      zo[:, :])

    nc.compile()
    return nc


def _pad_wout(w):
    # match oT_sb partition layout: [Z | h0 d (32) | ... | Z | h1 d (32)]
    wp = np.zeros((97, C_S), np.float32)
    wp[1:33] = w[0:32]
    wp[65:97] = w[32:64]
    return wp.astype(bf16)


def _host_prep(inputs):
    s1 = np.asarray(inputs["s1"], np.float32)
    s2 = np.asarray(inputs["s2"], np.float32)
    ridx1 = np.asarray(inputs["ridx1"], np.int32)
    ct1 = np.asarray(inputs["ct1"], np.int32)
    mask1 = np.asarray(inputs["mask1"], np.int32)
    mask2 = np.asarray(inputs["mask2"], np.int32)
    Wq = np.asarray(inputs["Wq"], np.float32)
    Wkv = np.asarray(inputs["Wkv"], np.float32)
    Wout = np.asarray(inputs["Wout"], np.float32)
    gq = np.asarray(inputs["gq"], np.float32)
    gk = np.asarray(inputs["gk"], np.float32)

    ct_idx = np.take_along_axis(ridx1, ct1[:, None], axis=1)
    pos = (ridx1 - ct_idx).astype(np.float32)
    half = C_S // 2
    freqs = np.exp(-np.log(10000.0) * np.arange(half, dtype=np.float32) / half)
    ang = pos[..., None] * freqs
    s1e = s1 + np.concatenate([np.sin(ang), np.cos(ang)], axis=-1).astype(np.float32)

    m1 = mask1.astype(np.float32)
    km = (mask2.astype(np.float32) - 1.0) * MASKV / SCALE

    g2 = gq * gk
    use_g2 = not np.allclose(g2, 1.0)

    ident = np.eye(128, dtype=bf16)
    in_maps = []
    for c in range(NCORES):
        b, hp = c // 4, c % 4
        m = {
            "s1T": np.ascontiguousarray(s1e[b].T).astype(bf16),
            "s2T": np.ascontiguousarray(s2[b].T).astype(bf16),
            "wq": np.ascontiguousarray(Wq[:, hp * HPC * D:(hp + 1) * HPC * D]).astype(bf16),
            "wkv": np.ascontiguousarray(Wkv[:, hp * HPC * 2 * D:(hp + 1) * HPC * 2 * D]).astype(bf16),
            "wout": _pad_wout(Wout[hp * HPC * D:(hp + 1) * HPC * D, :]),
            "qm": m1[b][None, :].astype(bf16),
            "km": km[b][None, :].astype(bf16),
            "ident": ident,
            "identf": np.ones((97, 1), dtype=bf16),
        }
        if use_g2:
            m["g2"] = np.tile(g2[None, hp * HPC * D:(hp + 1) * HPC * D], (128, 1)).astype(bf16)
        in_maps.append(m)
    return in_maps, use_g2, np.asarray(inputs["b_out"], np.float32)


def _run(inputs, trace=False, **kw):
    in_maps, use_g2, b_out = _host_prep(inputs)
    key = ("nc", use_g2)
    if key not in _cache:
        _cache[key] = _build(use_g2)
    nc = _cache[key]
    res = bass_utils.run_bass_kernel_spmd(
        nc, in_maps, core_ids=list(range(NCORES)), trace=trace, **kw)
    out = np.zeros((B, N1, C_S), np.float32)
    for c in range(NCORES):
        out[c // 4] += res.results[c]["out"]
    out += b_out[None, None, :]
    return out, res


def kernel(**inputs) -> np.ndarray:
    out, _ = _run(inputs, trace=False)
    return out


# revision 40
# speedup vs baseline: 1.2337x; 1.1818x over previous
"""Trainium2 Bass kernel for nn_Attention_12146167513140.

Distributed dense attention over 8 NeuronCores.

Sharding: core c in 0..7 -> (b = c//4, head-pair hp = c%4).  Each core
computes the full [3072 q x 3072 k] attention for its 2 heads of its
batch, producing a partial output projection [3072, 256]; the host sums
the 4 partials per batch and adds b_out.

Device pipeline per core (all matmuls bf16, accumulation f32 in PSUM):
  A) kv = s2 @ Wkv_pair -> rms-norm k -> kT2 (PE transpose),
     v -> vx tiles (4 sub-tiles of 17 cols: 16 v dims + ones col for Z)
  B) q = s1e @ Wq_pair -> rms-norm q -> qT2
  C) flash-style, PE-array tiled:
     - QK: 2-way row tiling (K=33: 32 dims + mask row).  Head 0 in array
       rows 0-63, head 1 in rows 64-127, concurrent.
     - exp: dispatched per key-chunk to ScalarE (exact exp, scale fused)
       or DVE via a 1-op Schraudolph fast-exp (f32 mult-add with int16
       writeback; the int16 bit pattern IS the bf16 exp approximation;
       RNE + saturation verified on HW).  Pattern "ADA" (period 3) pins
       one engine per sc-ring slot.  Mask bias is -30/SCALE so fast-exp
       inputs stay in the int16-safe range.
     - PV: 2-way column tiling (M=33 = ones|v per head); the leading
       ones column accumulates the softmax denominator Z at aligned
       PSUM partitions 0/64.
  D) 1/Z via tiny PE transposes of the Z rows already in oT_sb + DVE
     reciprocal; out_partial = (oT/Z).T @ Wout (zero-padded rows drop
     the Z row and match the oT_sb layout), 2-way row-tiled.

Host-side prep: sinusoidal positional embedding (index arithmetic),
transposes, bf16 casts, mask row encoding.
"""

import contextlib
import ctypes
import sys
import types

import numpy as np
import ml_dtypes

import concourse.bacc as bacc
import concourse.mybir as mybir
from concourse import bass_utils
from concourse.tile import TileContext
from concourse.alu_op_type import AluOpType
from concourse.mybir import ActivationFunctionType as AF


def _ensure_trace_support():
    """The container's antenv package lacks axon_hooks; bass_utils
    imports it when tracing is requested (e.g. via BASS_TRACE).  Install
    a functional shim so a traced run works instead of crashing, and
    make the artifact upload a no-op (no bucket access here)."""
    try:
        import antenv.axon_hooks  # noqa: F401
        return
    except ImportError:
        pass
    mod = types.ModuleType("antenv.axon_hooks")
    mod._hook = None
    mod.set_axon_ntff_profile_hook = lambda h: setattr(mod, "_hook", h)
    mod.get_axon_ntff_profile_hook = lambda: mod._hook
    try:
        import antenv
        sys.modules["antenv.axon_hooks"] = mod
        antenv.axon_hooks = mod
    except ImportError:
        sys.modules["antenv.axon_hooks"] = mod

    def _ntff_hook(so_path):
        try:
            lib = ctypes.CDLL(so_path)
        except OSError:
            return None
        if not hasattr(lib, "axon_start_nrt_profile"):
            return None
        lib.axon_start_nrt_profile.argtypes = [ctypes.POINTER(ctypes.c_int64),
                                               ctypes.c_size_t]
        lib.axon_start_nrt_profile.restype = ctypes.c_int64
        lib.axon_stop_nrt_profile.argtypes = [ctypes.c_char_p]
        lib.axon_stop_nrt_profile.restype = ctypes.c_int64

        @contextlib.contextmanager
        def _hook(output_dir, device_ids):
            import jax
            jax.devices()
            if device_ids:
                ids = (ctypes.c_int64 * len(device_ids))(*device_ids)
                rc = lib.axon_start_nrt_profile(ids, len(device_ids))
            else:
                rc = lib.axon_start_nrt_profile(None, 0)
            if rc != 0:
                raise RuntimeError(f"axon_start_nrt_profile rc={rc}")
            try:
                yield
            finally:
                lib.axon_stop_nrt_profile(str(output_dir).encode())

        return _hook

    mod.set_axon_ntff_profile_hook(_ntff_hook("/opt/axon/libaxon_pjrt.so"))

    _orig_upload = bass_utils.upload_artifacts

    def _safe_upload(tmpdir):
        try:
            return _orig_upload(tmpdir)
        except Exception:
            return tmpdir

    bass_utils.upload_artifacts = _safe_upload


_ensure_trace_support()

AX = mybir.AxisListType
I16 = mybir.dt.int16
I32 = mybir.dt.int32
BF = mybir.dt.bfloat16
F32 = mybir.dt.float32
bf16 = ml_dtypes.bfloat16

B, N1, N2 = 2, 3072, 3072
C_S, H, D = 256, 8, 32
INF = 100000.0
EPS = 1e-8
SCALE = float(np.sqrt(1.0 / (3 * D)))
MASKV = 30.0          # effective -30 in the exp argument for masked keys

NCORES = 8
HPC = 2            # heads per core
KCH = N2 // 128    # 24 key chunks
QCH = N1 // 128    # 24 q row chunks
QB = 512           # q block for scores free dim
NQB = N1 // QB     # 6
VJ = 17            # PV col-tile width: 16 v dims + ones col

# Schraudolph fast-exp constants: int16(y*AS + BS) viewed as bf16 ~ exp(y)
# for y = raw score (SCALE folded into AS).  Calibrated on HW (RNE writeback).
LN2 = float(np.log(2.0))
AS_EXP = SCALE * 128.0 / LN2
BS_EXP = 127.0 * 128.0 - 7.4

import os
# exp engine dispatch patterns (per key chunk index):
# A = ScalarE exact exp, D = DVE fast-exp (GPSIMD cannot read PSUM)
PAT_EARLY = os.environ.get("KPE", "A")    # qb 0-1: ScalarE only; DVE runs prologue fillers
PAT_MAIN = os.environ.get("KPM", "ADA")   # qb 2+: period 3 = one engine per ring slot

_cache = {}
KBISECT = int(os.environ.get("KBISECT", "5"))


def _build(use_g2: bool):
    nc = bacc.Bacc("TRN2", target_bir_lowering=False, debug=False, num_devices=NCORES)

    s1T_d = nc.dram_tensor("s1T", [C_S, N1], BF, kind="ExternalInput")
    s2T_d = nc.dram_tensor("s2T", [C_S, N2], BF, kind="ExternalInput")
    wq_d = nc.dram_tensor("wq", [C_S, HPC * D], BF, kind="ExternalInput")
    wkv_d = nc.dram_tensor("wkv", [C_S, HPC * 2 * D], BF, kind="ExternalInput")
    wout_d = nc.dram_tensor("wout", [97, C_S], BF, kind="ExternalInput")
    qm_d = nc.dram_tensor("qm", [1, N1], BF, kind="ExternalInput")
    km_d = nc.dram_tensor("km", [1, N2], BF, kind="ExternalInput")
    id_d = nc.dram_tensor("ident", [128, 128], BF, kind="ExternalInput")
    idf_d = nc.dram_tensor("identf", [97, 1], BF, kind="ExternalInput")
    if use_g2:
        g2_d = nc.dram_tensor("g2", [128, HPC * D], BF, kind="ExternalInput")
    out_d = nc.dram_tensor("out", [N1, C_S], F32, kind="ExternalOutput")

    with TileContext(nc) as tc:
        with (
            tc.tile_pool(name="const", bufs=1) as cpool,
            tc.tile_pool(name="work", bufs=4) as work,
            tc.tile_pool(name="norm", bufs=6) as npool,
            tc.tile_pool(name="expp", bufs=6) as expp,
            tc.tile_pool(name="psR", bufs=3, space="PSUM") as psR,
            tc.tile_pool(name="psO", bufs=1, space="PSUM") as psO,
            tc.tile_pool(name="psM", bufs=1, space="PSUM") as psM,
        ):
            # ---- constants / staging ----
            ident = cpool.tile([128, 128], BF)
            nc.sync.dma_start(ident[:, :], id_d.ap())
            identf = cpool.tile([97, 1], BF, tag="identf")
            nc.sync.dma_start(identf[:, :], idf_d.ap())

            wq_sb = cpool.tile([128, HPC * D], BF, tag="wq")
            wq_sb2 = cpool.tile([128, HPC * D], BF, tag="wq2")
            nc.sync.dma_start(wq_sb[:, :], wq_d.ap()[0:128, :])
            nc.sync.dma_start(wq_sb2[:, :], wq_d.ap()[128:256, :])
            wkv_sb = cpool.tile([128, HPC * 2 * D], BF, tag="wkv")
            wkv_sb2 = cpool.tile([128, HPC * 2 * D], BF, tag="wkv2")
            nc.sync.dma_start(wkv_sb[:, :], wkv_d.ap()[0:128, :])
            nc.sync.dma_start(wkv_sb2[:, :], wkv_d.ap()[128:256, :])
            wout_sb = cpool.tile([97, C_S], BF, tag="wout")
            nc.sync.dma_start(wout_sb[:, :], wout_d.ap())
            if use_g2:
                g2_sb = cpool.tile([128, HPC * D], BF, tag="g2")
                nc.sync.dma_start(g2_sb[:, :], g2_d.ap())

            s1T = [cpool.tile([128, N1], BF, tag=f"s1T{i}", name=f"s1T{i}") for i in range(2)]
            s2T = [cpool.tile([128, N2], BF, tag=f"s2T{i}", name=f"s2T{i}") for i in range(2)]
            for j in range(8):
                sl = slice(j * (N1 // 8), (j + 1) * (N1 // 8))
                for i in range(2):
                    nc.sync.dma_start(s2T[i][:, sl], s2T_d.ap()[i * 128:(i + 1) * 128, sl])
                    nc.sync.dma_start(s1T[i][:, sl], s1T_d.ap()[i * 128:(i + 1) * 128, sl])

            # two-head packed transposed q/k: head0 dims in rows 0-31 with
            # mask row 32, head1 dims in rows 64-95 with mask row 96.  Rows
            # 33-63 / 97-127 are never streamed (K=33 APs).
            kT2 = cpool.tile([128, N2], BF, tag="kT2", name="kT2")
            qT2 = cpool.tile([128, N1], BF, tag="qT2", name="qT2")
            nc.sync.dma_start(kT2[32:33, :], km_d.ap())
            nc.sync.dma_start(kT2[96:97, :], km_d.ap())
            nc.sync.dma_start(qT2[32:33, :], qm_d.ap())
            nc.sync.dma_start(qT2[96:97, :], qm_d.ap())

            # v-extended: per kchunk 66 cols, 2 col-tiles of M=33:
            # [ones|v_h0 (32)][ones|v_h1 (32)]
            # ones lead so Z lands at oT partitions 0/64 (aligned).
            vx = cpool.tile([128, KCH * 66], BF, tag="vx")
            vx4 = vx[:, :].rearrange("p (k h x) -> p k h x", h=2, x=33)
            nc.vector.memset(vx4[:, :, :, 0:1], 1.0)

            oT_sb = cpool.tile([128, N1], BF, tag="oT")
            nc.vector.memset(oT_sb[:, :], 0.0)
            rzp = cpool.tile([128, QCH * HPC], F32, tag="rzp")

            # ---- phases A/B: projections + rms-norm + transposes ----
            NCHUNK = KCH + QCH  # 48
            kcp_all = cpool.tile([128, NCHUNK * HPC * D], BF, tag="kcp_all")
            ss_all = cpool.tile([128, NCHUNK * HPC], F32, tag="ss_all")

            def pass1_g(kcs, sT, w1, w2, kvside):
                # 4 chunks per PSUM bank: one kcp copy / sq / reduce / v-copy
                ng = len(kcs)
                ncol = w1.shape[1]
                ci0 = kcs[0] if kvside else KCH + kcs[0]
                pp = psM.tile([128, 4 * ncol], F32, tag="mx", name=f"pp{ci0}")
                for idx, kc in enumerate(kcs):
                    nc.tensor.matmul(pp[:, idx * ncol:(idx + 1) * ncol],
                                     sT[0][:, kc * 128:(kc + 1) * 128], w1[:, :],
                                     start=True, stop=False)
                    nc.tensor.matmul(pp[:, idx * ncol:(idx + 1) * ncol],
                                     sT[1][:, kc * 128:(kc + 1) * 128], w2[:, :],
                                     start=False, stop=True)
                kcp = kcp_all[:, ci0 * HPC * D:(ci0 + ng) * HPC * D]
                nc.vector.tensor_copy(
                    kcp.rearrange("p (c h d) -> p c h d", c=ng, d=D),
                    pp[:, 0:ng * ncol].rearrange("p (c h x) -> p c h x",
                                                 c=ng, h=HPC)[:, :, :, 0:D])
                sq = npool.tile([128, 4 * HPC * D], F32, tag="sq",
                                name=f"sq{ci0}")
                nc.gpsimd.tensor_tensor(sq[:, 0:ng * HPC * D], kcp, kcp,
                                        AluOpType.mult)
                nc.vector.reduce_sum(
                    ss_all[:, ci0 * HPC:(ci0 + ng) * HPC],
                    sq[:, 0:ng * HPC * D].rearrange("p (ch d) -> p ch d", d=D),
                    axis=AX.X)
                if kvside:  # copy v into vx cols 1-33 per head (+cast bf16)
                    nc.vector.tensor_copy(
                        vx[:, kcs[0] * 66:(kcs[0] + ng) * 66]
                        .rearrange("p (c h x) -> p c h x", c=ng, x=33)[:, :, :, 1:33],
                        pp[:, 0:ng * ncol].rearrange("p (c h x) -> p c h x",
                                                     c=ng, h=HPC)[:, :, :, D:2 * D])

            sr_all = cpool.tile([128, NCHUNK * HPC], F32, tag="sr_all")
            rinv_all = cpool.tile([128, NCHUNK * HPC], BF, tag="rinv_all")

            def rsqrt_batch(sl, bid):
                # rinv = 1/sqrt(ss/D + eps) entirely on DVE (bit-trick seed +
                # 2 Newton steps, ~5e-6 rel err).  Keeps Sqrt off ScalarE so
                # the in-order ACT stream carries nothing but Exp ops.
                w = sl.stop - sl.start
                x = sr_all[:, sl]
                nc.vector.tensor_scalar(x, ss_all[:, sl], 1.0 / D, EPS,
                                        AluOpType.mult, AluOpType.add)
                t = npool.tile([128, NCHUNK * HPC], I32, tag="nrt",
                               name=f"nrt{bid}")
                nc.vector.tensor_scalar(t[:, 0:w], x.bitcast(I32), 1, None,
                                        AluOpType.arith_shift_right)
                u = npool.tile([128, NCHUNK * HPC], I32, tag="nru",
                               name=f"nru{bid}")
                nc.vector.tensor_scalar(u[:, 0:w], t[:, 0:w], -1, 0x5F3759DF,
                                        AluOpType.mult, AluOpType.add)
                y = u[:, 0:w].bitcast(F32)
                for it in range(2):
                    a = npool.tile([128, NCHUNK * HPC], F32, tag="nra",
                                   name=f"nra{bid}_{it}")
                    nc.vector.tensor_tensor(a[:, 0:w], y, y, AluOpType.mult)
                    b = npool.tile([128, NCHUNK * HPC], F32, tag="nrb",
                                   name=f"nrb{bid}_{it}")
                    nc.vector.tensor_tensor(b[:, 0:w], a[:, 0:w], x,
                                            AluOpType.mult)
                    c = npool.tile([128, NCHUNK * HPC], F32, tag="nrc",
                                   name=f"nrc{bid}_{it}")
                    nc.vector.tensor_scalar(c[:, 0:w], b[:, 0:w], -0.5, 1.5,
                                            AluOpType.mult, AluOpType.add)
                    if it == 0:
                        dst = npool.tile([128, NCHUNK * HPC], F32, tag="nry",
                                         name=f"nry{bid}_{it}")
                        out = dst[:, 0:w]
                    else:
                        out = rinv_all[:, sl]  # bf16 writeback on last step
                    nc.vector.tensor_tensor(out, y, c[:, 0:w], AluOpType.mult)
                    y = out

            def pass2(ci, kc, dstT, qside):
                kcp = kcp_all[:, ci * HPC * D:(ci + 1) * HPC * D]
                pre = npool.tile([128, HPC * D], BF, tag="pre", name=f"pre{ci}")
                rb = rinv_all[:, ci * HPC:(ci + 1) * HPC]
                nc.gpsimd.tensor_tensor(
                    pre[:, :].rearrange("p (h d) -> p h d", d=D),
                    kcp.rearrange("p (h d) -> p h d", d=D),
                    rb[:, :, None].broadcast_to([128, HPC, D]),
                    AluOpType.mult)
                if use_g2 and qside:  # q side carries the gq*gk factor
                    nc.vector.tensor_tensor(pre[:, :], pre[:, :], g2_sb[:, :],
                                            AluOpType.mult)
                tp = psM.tile([HPC * D, 128], BF, tag="mx", name=f"tp{ci}")
                nc.tensor.transpose(tp[:, :], pre[:, :], ident[:, :])
                nc.vector.tensor_copy(dstT[0:32, kc * 128:(kc + 1) * 128],
                                      tp[0:32, :])
                nc.vector.tensor_copy(dstT[64:96, kc * 128:(kc + 1) * 128],
                                      tp[32:64, :])

            def norm_batch(chunks, kvside):
                for g0 in range(0, len(chunks), 4):
                    kcs = chunks[g0:g0 + 4]
                    if kvside:
                        pass1_g(kcs, s2T, wkv_sb, wkv_sb2, True)
                    else:
                        pass1_g(kcs, s1T, wq_sb, wq_sb2, False)
                ci0 = (chunks[0] if kvside else KCH + chunks[0]) * HPC
                ci1 = (chunks[-1] if kvside else KCH + chunks[-1]) * HPC + HPC
                rsqrt_batch(slice(ci0, ci1), f"b{ci0}")
                for kc in chunks:
                    if kvside:
                        pass2(kc, kc, kT2, False)
                    else:
                        pass2(KCH + kc, kc, qT2, True)

            oT_tiles = {}
            pending_drain = []

            def drain(qb):
                # 1/Z from the Z rows already sitting in oT_sb (parts 0/64)
                ztr = psM.tile([128, 8 * HPC], BF, tag="mx", name=f"ztr{qb}")
                for c in range(4):
                    for h in range(HPC):
                        i = c * HPC + h
                        nc.tensor.transpose(
                            ztr[:, 2 * i:2 * i + 1],
                            oT_sb[h * 64:h * 64 + 1,
                                  qb * QB + c * 128:qb * QB + (c + 1) * 128],
                            identf[h * 64:h * 64 + 1, 0:1])
                nc.vector.reciprocal(
                    rzp[:, qb * 4 * HPC:(qb + 1) * 4 * HPC],
                    ztr[:, :].rearrange("p (i two) -> p i two", two=2)[:, :, 0])

            def attend(qb, fillers=(), pattern=PAT_MAIN):
                fillers = list(fillers)
                fillers[1:1] = pending_drain
                del pending_drain[:]
                nf = len(fillers)
                qsl = slice(qb * QB, (qb + 1) * QB)
                oT = None
                for kc in range(KCH):
                    while fillers and (nf - len(fillers)) * KCH <= kc * nf:
                        fillers.pop(0)()
                    if oT is None:
                        oT = psO.tile([128, QB], F32, tag="oT", name=f"oT_{qb}")
                        oT_tiles[qb] = oT
                    sc = psR.tile([128, HPC * QB], F32, tag="sc",
                                  name=f"sc_{qb}_{kc}")
                    nc.tensor.matmul(
                        sc[:, 0:QB],
                        kT2[0:33, kc * 128:(kc + 1) * 128],
                        qT2[0:33, qsl], start=True, stop=True)
                    nc.tensor.matmul(
                        sc[:, QB:2 * QB],
                        kT2[64:97, kc * 128:(kc + 1) * 128],
                        qT2[64:97, qsl], start=True, stop=True)
                    ex = expp.tile([128, HPC * QB], BF, tag="ex",
                                   name=f"ex_{qb}_{kc}")
                    eng = pattern[kc % len(pattern)]
                    if eng == "A":
                        nc.scalar.activation(ex[:, :], sc[:, :], AF.Exp,
                                             scale=SCALE)
                    elif eng == "D":
                        nc.vector.tensor_scalar(
                            ex[:, :].bitcast(I16), sc[:, :], AS_EXP, BS_EXP,
                            AluOpType.mult, AluOpType.add)
                    else:
                        nc.gpsimd.tensor_scalar(
                            ex[:, :].bitcast(I16), sc[:, :], AS_EXP, BS_EXP,
                            AluOpType.mult, AluOpType.add)
                    for j in range(2):
                        nc.tensor.matmul(
                            oT[j * 64:j * 64 + 33, :],
                            vx[:, kc * 66 + j * 33:kc * 66 + (j + 1) * 33],
                            ex[:, j * QB:(j + 1) * QB],
                            start=(kc == 0), stop=(kc == KCH - 1),
                            tile_position=(0, j * 64))
                nc.vector.tensor_copy(oT_sb[0:33, qsl], oT[0:33, :])
                nc.vector.tensor_copy(oT_sb[64:97, qsl], oT[64:97, :])
                oT_tiles.pop(qb, None)
                if os.environ.get("KDEFER", "1") == "1":
                    pending_drain.append(lambda: drain(qb))
                else:
                    drain(qb)

            def proj_out(qc):
                osl = slice(qc * 128, (qc + 1) * 128)
                op0 = psM.tile([128, C_S], F32, tag="mx", name=f"op0_{qc}")
                nc.tensor.matmul(op0[:, :], oT_sb[0:33, osl], wout_sb[0:33, :],
                                 start=True, stop=True)
                op1 = psM.tile([128, C_S], F32, tag="mx", name=f"op1_{qc}")
                nc.tensor.matmul(op1[:, :], oT_sb[64:97, osl],
                                 wout_sb[64:97, :], start=True, stop=True)
                t0 = work.tile([128, C_S], F32, tag="t0", name=f"t0_{qc}")
                nc.vector.tensor_scalar(t0[:, :], op0[:, :],
                                        rzp[:, qc * HPC:qc * HPC + 1], None,
                                        AluOpType.mult)
                ops = work.tile([128, C_S], F32, tag="osb", name=f"osb_{qc}")
                nc.vector.scalar_tensor_tensor(
                    ops[:, :], op1[:, :], rzp[:, qc * HPC + 1:qc * HPC + 2],
                    t0[:, :], AluOpType.mult, AluOpType.add)
                nc.sync.dma_start(out_d.ap()[osl, :], ops[:, :])

            # prologue schedule: attend(0) needs qT2 chunks 0-3 and kT2
            # incrementally; emit the minimum before it and overlap the rest.
            # The out-projection for q-block qb is emitted after attend(qb+1)
            # so its matmuls fill PE bubbles instead of forming a tail.
            norm_batch(list(range(0, 8)), kvside=False)
            norm_batch(list(range(0, 8)), kvside=True)
            norm_batch(list(range(8, 16)), kvside=True)
            norm_batch(list(range(16, 24)), kvside=True)
            if KBISECT >= 2:
                f0 = [(lambda g0=g0: pass1_g(list(range(g0, g0 + 2)),
                                             s1T, wq_sb, wq_sb2, False))
                      for g0 in range(8, 24, 2)]
                attend(0, f0, PAT_EARLY)
                q_sl2 = slice((KCH + 8) * HPC, (KCH + 24) * HPC)
                rsqrt_batch(q_sl2, "q2")
            if KBISECT >= 3:
                f1 = [(lambda kc=kc: pass2(KCH + kc, kc, qT2, True))
                      for kc in range(8, 24)]
                attend(1, f1, PAT_EARLY)
            if KBISECT >= 4:
                for qb in range(2, NQB):
                    hi = (qb - 1) * 4 + (4 if qb == NQB - 1 else 0)
                    fd = ([(lambda qc=qc: proj_out(qc))
                           for qc in range((qb - 2) * 4, hi)]
                          if KBISECT >= 5 else [])
                    attend(qb, fd, PAT_MAIN)
            if KBISECT >= 5:
                for d in pending_drain:
                    d()
                del pending_drain[:]
                for qc in range((NQB - 1) * 4, NQB * 4):
                    proj_out(qc)
            else:
                zo = work.tile([128, C_S], F32, tag="osb")
                nc.vector.memset(zo[:, :], 0.0)
                for qc in range(QCH):
                    nc.sync.dma_start(out_d.ap()[qc * 128:(qc + 1) * 128, :],
                                      zo[:, :])

    nc.compile()
    return nc


def _pad_wout(w):
    # match oT_sb partition layout: [Z | h0 d (32) | ... | Z | h1 d (32)]
    wp = np.zeros((97, C_S), np.float32)
    wp[1:33] = w[0:32]
    wp[65:97] = w[32:64]
    return wp.astype(bf16)


def _host_prep(inputs):
    s1 = np.asarray(inputs["s1"], np.float32)
    s2 = np.asarray(inputs["s2"], np.float32)
    ridx1 = np.asarray(inputs["ridx1"], np.int32)
    ct1 = np.asarray(inputs["ct1"], np.int32)
    mask1 = np.asarray(inputs["mask1"], np.int32)
    mask2 = np.asarray(inputs["mask2"], np.int32)
    Wq = np.asarray(inputs["Wq"], np.float32)
    Wkv = np.asarray(inputs["Wkv"], np.float32)
    Wout = np.asarray(inputs["Wout"], np.float32)
    gq = np.asarray(inputs["gq"], np.float32)
    gk = np.asarray(inputs["gk"], np.float32)

    ct_idx = np.take_along_axis(ridx1, ct1[:, None], axis=1)
    pos = (ridx1 - ct_idx).astype(np.float32)
    half = C_S // 2
    freqs = np.exp(-np.log(10000.0) * np.arange(half, dtype=np.float32) / half)
    ang = pos[..., None] * freqs
    s1e = s1 + np.concatenate([np.sin(ang), np.cos(ang)], axis=-1).astype(np.float32)

    m1 = mask1.astype(np.float32)
    km = (mask2.astype(np.float32) - 1.0) * MASKV / SCALE

    g2 = gq * gk
    use_g2 = not np.allclose(g2, 1.0)

    ident = np.eye(128, dtype=bf16)
    in_maps = []
    for c in range(NCORES):
        b, hp = c // 4, c % 4
        m = {
            "s1T": np.ascontiguousarray(s1e[b].T).astype(bf16),
            "s2T": np.ascontiguousarray(s2[b].T).astype(bf16),
            "wq": np.ascontiguousarray(Wq[:, hp * HPC * D:(hp + 1) * HPC * D]).astype(bf16),
            "wkv": np.ascontiguousarray(Wkv[:, hp * HPC * 2 * D:(hp + 1) * HPC * 2 * D]).astype(bf16),
            "wout": _pad_wout(Wout[hp * HPC * D:(hp + 1) * HPC * D, :]),
            "qm": m1[b][None, :].astype(bf16),
            "km": km[b][None, :].astype(bf16),
            "ident": ident,
            "identf": np.ones((97, 1), dtype=bf16),
        }
        if use_g2:
            m["g2"] = np.tile(g2[None, hp * HPC * D:(hp + 1) * HPC * D], (128, 1)).astype(bf16)
        in_maps.append(m)
    return in_maps, use_g2, np.asarray(inputs["b_out"], np.float32)


def _run(inputs, trace=False, **kw):
    in_maps, use_g2, b_out = _host_prep(inputs)
    key = ("nc", use_g2)
    if key not in _cache:
        _cache[key] = _build(use_g2)
    nc = _cache[key]
    res = bass_utils.run_bass_kernel_spmd(
        nc, in_maps, core_ids=list(range(NCORES)), trace=trace, **kw)
    out = np.zeros((B, N1, C_S), np.float32)
    for c in range(NCORES):
        out[c // 4] += res.results[c]["out"]
    out += b_out[None, None, :]
    return out, res


def kernel(**inputs) -> np.ndarray:
    out, _ = _run(inputs, trace=False)
    return out
